# revision 7
# baseline (speedup 1.0000x reference)
"""Trainium2 Bass kernel for the VQ-VAE codebook-EMA module.

Data-parallel over 8 NeuronCores: each core handles 4 of the 32 batch
images (4096 of the 32768 latent vectors), the [K, D] codebook is
replicated, and the per-device one-hot counts / dw = encodings^T z /
loss partials are combined with an on-device AllReduce. Every core then
runs the identical (tiny) EMA finalization so core 0's copy of the
updated codebook tensors is the full answer.
"""
import sys

if '/opt/trn_rl_repo' not in sys.path:
    sys.path.insert(0, '/opt/trn_rl_repo')

import numpy as np

import concourse.bass as bass
import concourse.bacc as bacc
import concourse.bass_isa as bass_isa
import concourse.mybir as mybir
import concourse.tile as tile
from concourse.bass_utils import run_bass_kernel_spmd
from concourse.masks import make_identity

# Problem constants (hardcoded per the harness contract).
B = 32
D = 256            # latent dim == channel dim
HW = 1024          # 32 * 32
K = 1024           # codebook size
BETA = 0.25
DECAY = 0.99
EPS = 1e-5
N_CORES = 8
B_LOC = B // N_CORES          # batch images per core
N_LOC = B_LOC * HW            # latent vectors per core
NT = HW // 128                # 128-row n-tiles per batch image
KC = K // 128                 # 128-wide k-chunks for dw
F32 = mybir.dt.float32
BF16 = mybir.dt.bfloat16
I32 = mybir.dt.int32
U32 = mybir.dt.uint32

_CACHED_NC = None


def _build_nc():
    nc = bacc.Bacc("TRN2", target_bir_lowering=False, debug=False)

    z_d = nc.declare_dram_parameter("z", [B_LOC, D, HW], F32, isOutput=False)
    emb_d = nc.declare_dram_parameter("emb", [K, D], F32, isOutput=False)
    ema_cs_d = nc.declare_dram_parameter("ema_cs", [K], F32, isOutput=False)
    ema_w_d = nc.declare_dram_parameter("ema_w", [K, D], F32, isOutput=False)

    zq_d = nc.declare_dram_parameter("zq", [B_LOC, D, HW], F32, isOutput=True)
    idx_d = nc.declare_dram_parameter("idx", [N_LOC, 1], I32, isOutput=True)
    loss_d = nc.declare_dram_parameter("loss", [1, 1], F32, isOutput=True)
    new_emb_d = nc.declare_dram_parameter("new_emb", [K, D], F32, isOutput=True)
    new_cs_d = nc.declare_dram_parameter("new_cs", [K], F32, isOutput=True)
    new_ema_w_d = nc.declare_dram_parameter("new_ema_w", [K, D], F32, isOutput=True)

    with tile.TileContext(nc) as tc:
        with (
            tc.tile_pool(name="persist", bufs=1) as pp,
            tc.tile_pool(name="dram", bufs=1, space="DRAM") as dramp,
        ):
            # ---------------- setup: e tiles, eT, bias ----------------
            ident = pp.tile([128, 128], F32)
            make_identity(nc, ident[:])

            e_km = []
            for t in range(KC):
                et = pp.tile([128, D], F32, name=f"e_km{t}", tag=f"e_km{t}")
                nc.sync.dma_start(et[:], emb_d[t * 128:(t + 1) * 128, :])
                e_km.append(et)

            # eT[d-chunk] : [128(d), K] fp32, via PE transposes of e_km
            eT = [pp.tile([128, K], F32, name=f"eT{d}", tag=f"eT{d}") for d in range(2)]
            halfones = pp.tile([128, 128], F32)
            nc.gpsimd.memset(halfones[:], 0.5)
            iota_k = pp.tile([128, K], F32)
            nc.gpsimd.iota(iota_k[:], pattern=[[1, K]], base=0, channel_multiplier=0,
                           allow_small_or_imprecise_dtypes=True)
            ones_col = pp.tile([128, 1], BF16)
            nc.gpsimd.memset(ones_col[:], 1.0)
            bias_sb = pp.tile([128, K], F32)

            with (
                tc.tile_pool(name="setup_ps", bufs=2, space="PSUM") as sps,
                tc.tile_pool(name="setup_sb", bufs=1) as ssb,
            ):
                for t in range(KC):
                    for d in range(2):
                        tp = sps.tile([128, 128], F32, tag="etr")
                        nc.tensor.transpose(tp[:], e_km[t][:, d * 128:(d + 1) * 128], ident[:])
                        nc.scalar.copy(eT[d][:, t * 128:(t + 1) * 128], tp[:])
                # esq = eT * eT ; bias = 0.5 * sum_d e^2 broadcast on partitions
                esq = [ssb.tile([128, K], F32, name=f"esq{d}", tag=f"esq{d}") for d in range(2)]
                for d in range(2):
                    nc.vector.tensor_tensor(out=esq[d][:], in0=eT[d][:], in1=eT[d][:],
                                            op=mybir.AluOpType.mult)
                for kc2 in range(2):
                    bp = sps.tile([128, 512], F32, tag="biasps")
                    sl = slice(kc2 * 512, (kc2 + 1) * 512)
                    nc.tensor.matmul(bp[:], halfones[:], esq[0][:, sl], start=True, stop=False)
                    nc.tensor.matmul(bp[:], halfones[:], esq[1][:, sl], start=False, stop=True)
                    nc.scalar.copy(bias_sb[:, sl], bp[:])

            # ---------------- persistent accumulators ----------------
            # dw_sb[kc] : [128(k), 257] fp32 (cols 0:256 dw, col 256 counts)
            dw_sb = [pp.tile([128, 257], F32, name=f"dw_sb{t}", tag=f"dw_sb{t}")
                     for t in range(KC)]
            for t in range(KC):
                nc.vector.memset(dw_sb[t][:], 0.0)
            zsq_cols = pp.tile([128, 8], F32)      # per-(b,d) z^2 partial sums
            mx_cols = pp.tile([128, B_LOC * NT], F32)

            # ---------------- main loop pools ----------------
            with (
                tc.tile_pool(name="zpool", bufs=2) as zp,
                tc.tile_pool(name="work", bufs=2) as wp,
                tc.tile_pool(name="ebig", bufs=2 * NT) as ep,
                tc.tile_pool(name="scores_ps", bufs=2, space="PSUM") as ps_s,
                tc.tile_pool(name="tr_ps", bufs=1, space="PSUM") as ps_t,
                tc.tile_pool(name="dw_ps", bufs=2, space="PSUM") as ps_dw,
                tc.tile_pool(name="zqout", bufs=2) as qp,
            ):
                zsq_ct = 0
                for b in range(B_LOC):
                    zT = []
                    for d in range(2):
                        zt = zp.tile([128, HW], F32, name=f"zT{d}", tag=f"zT{d}")
                        nc.sync.dma_start(zt[:], z_d[b, d * 128:(d + 1) * 128, :])
                        zT.append(zt)

                    # z^2 partial sums on the scalar engine (Square + accum)
                    for d in range(2):
                        junk = wp.tile([128, HW], BF16, tag="sq_junk")
                        nc.scalar.activation(
                            out=junk[:], in_=zT[d][:],
                            func=mybir.ActivationFunctionType.Square,
                            accum_out=zsq_cols[:, zsq_ct:zsq_ct + 1],
                        )
                        zsq_ct += 1

                    zqT = [qp.tile([128, HW], F32, name=f"zqT{d}", tag=f"zqT{d}")
                           for d in range(2)]

                    E_tiles, hi_tiles, lo_tiles = [], [], []
                    for i in range(NT):
                        nsl = slice(i * 128, (i + 1) * 128)
                        # ---- scores: S[n, k] = z . e  (fp32) ----
                        s0 = ps_s.tile([128, 512], F32, tag="s0")
                        s1 = ps_s.tile([128, 512], F32, tag="s1")
                        nc.tensor.matmul(s0[:], zT[0][:, nsl], eT[0][:, 0:512], start=True, stop=False)
                        nc.tensor.matmul(s1[:], zT[0][:, nsl], eT[0][:, 512:1024], start=True, stop=False)
                        nc.tensor.matmul(s0[:], zT[1][:, nsl], eT[1][:, 0:512], start=False, stop=True)
                        nc.tensor.matmul(s1[:], zT[1][:, nsl], eT[1][:, 512:1024], start=False, stop=True)
                        s_sb = wp.tile([128, K], F32, tag="s_sb")
                        nc.scalar.copy(s_sb[:, 0:512], s0[:])
                        nc.scalar.copy(s_sb[:, 512:1024], s1[:])
                        # score - 0.5*||e||^2
                        nc.gpsimd.tensor_tensor(out=s_sb[:], in0=s_sb[:], in1=bias_sb[:],
                                                op=mybir.AluOpType.subtract)
                        # ---- argmax over k ----
                        mx = wp.tile([128, 8], F32, tag="mx")
                        mi = wp.tile([128, 8], U32, tag="mi")
                        nc.vector.max(out=mx[:], in_=s_sb[:])
                        nc.vector.max_index(out=mi[:], in_max=mx[:], in_values=s_sb[:])
                        col = b * NT + i
                        nc.vector.tensor_copy(mx_cols[:, col:col + 1], mx[:, 0:1])
                        nc.sync.dma_start(idx_d[col * 128:(col + 1) * 128, :],
                                          mi[:, 0:1].bitcast(I32))
                        idxf = wp.tile([128, 1], F32, tag="idxf")
                        nc.vector.tensor_copy(idxf[:], mi[:, 0:1].bitcast(I32))
                        # ---- one-hot row E[n, k] (bf16, exact) ----
                        E_t = ep.tile([128, K], BF16, tag="E")
                        nc.gpsimd.tensor_scalar(
                            out=E_t[:], in0=iota_k[:], scalar1=idxf[:, 0:1], scalar2=None,
                            op0=mybir.AluOpType.is_equal)
                        E_tiles.append(E_t)
                        # ---- z_q gather (exact fp32 rows of emb) ----
                        zq_nm = wp.tile([128, D], F32, tag="zq_nm")
                        nc.gpsimd.indirect_dma_start(
                            out=zq_nm[:], out_offset=None,
                            in_=emb_d[:],
                            in_offset=bass.IndirectOffsetOnAxis(ap=mi[:, 0:1].bitcast(I32), axis=0),
                        )
                        # ---- transposes: zf (for dw rhs) and zq (for output) ----
                        tzf = ps_t.tile([128, D], F32, tag="tzf")
                        nc.tensor.transpose(tzf[:, 0:128], zT[0][:, nsl], ident[:])
                        nc.tensor.transpose(tzf[:, 128:256], zT[1][:, nsl], ident[:])
                        hi = ep.tile([128, D + 1], BF16, tag="hi")
                        nc.scalar.copy(hi[:, 0:D], tzf[:])
                        nc.gpsimd.memset(hi[:, D:D + 1], 1.0)
                        lo = ep.tile([128, D], BF16, tag="lo")
                        nc.vector.tensor_tensor(out=lo[:], in0=tzf[:], in1=hi[:, 0:D],
                                                op=mybir.AluOpType.subtract)
                        hi_tiles.append(hi)
                        lo_tiles.append(lo)

                        tzq = ps_t.tile([128, D], F32, tag="tzq")
                        nc.tensor.transpose(tzq[:, 0:128], zq_nm[:, 0:128], ident[:])
                        nc.tensor.transpose(tzq[:, 128:256], zq_nm[:, 128:256], ident[:])
                        nc.scalar.copy(zqT[0][:, nsl], tzq[:, 0:128])
                        nc.scalar.copy(zqT[1][:, nsl], tzq[:, 128:256])

                    # ---- dw accumulation, k-chunk outer, hi+lo in one group ----
                    for t in range(KC):
                        ksl = slice(t * 128, (t + 1) * 128)
                        dp = ps_dw.tile([128, 257], F32, tag="dwps")
                        for i in range(NT):
                            nc.tensor.matmul(dp[:, 0:257], E_tiles[i][:, ksl],
                                             hi_tiles[i][:], start=(i == 0), stop=False)
                        for i in range(NT):
                            nc.tensor.matmul(dp[:, 0:256], E_tiles[i][:, ksl],
                                             lo_tiles[i][:], start=False,
                                             stop=(i == NT - 1))
                        nc.vector.tensor_tensor(out=dw_sb[t][:], in0=dw_sb[t][:],
                                                in1=dp[:], op=mybir.AluOpType.add)

                    for d in range(2):
                        nc.sync.dma_start(zq_d[b, d * 128:(d + 1) * 128, :], zqT[d][:])

            # ---------------- loss partial + all-reduce ----------------
            with tc.tile_pool(name="fin", bufs=1) as fp:
                zsq_sum = fp.tile([128, 1], F32)
                nc.vector.reduce_sum(out=zsq_sum[:], in_=zsq_cols[:], axis=mybir.AxisListType.X)
                zsq_par = fp.tile([128, 1], F32)
                nc.gpsimd.partition_all_reduce(zsq_par[:], zsq_sum[:],
                                               channels=128, reduce_op=bass_isa.ReduceOp.add)
                mx_sum = fp.tile([128, 1], F32)
                nc.vector.reduce_sum(out=mx_sum[:], in_=mx_cols[:], axis=mybir.AxisListType.X)
                mx_par = fp.tile([128, 1], F32)
                nc.gpsimd.partition_all_reduce(mx_par[:], mx_sum[:],
                                               channels=128, reduce_op=bass_isa.ReduceOp.add)
                loss_par = fp.tile([128, 1], F32)
                nc.vector.tensor_scalar(out=loss_par[:], in0=mx_par[:], scalar1=-2.0,
                                        scalar2=zsq_par[:, 0:1],
                                        op0=mybir.AluOpType.mult, op1=mybir.AluOpType.add)

                ar_in = dramp.tile([K, 258], F32)
                ar_out = dramp.tile([K, 258], F32, addr_space="Shared")
                for t in range(KC):
                    nc.sync.dma_start(ar_in[t * 128:(t + 1) * 128, 0:257], dw_sb[t][:])
                nc.sync.dma_start(
                    ar_in[:, 257:258].rearrange("(t p) c -> p (t c)", p=128),
                    loss_par[:, 0:1].to_broadcast([128, KC]))
                nc.gpsimd.collective_compute(
                    "AllReduce", mybir.AluOpType.add,
                    replica_groups=[list(range(N_CORES))],
                    ins=[ar_in[:]], outs=[ar_out[:]],
                )

                # ---------------- EMA finalization (identical on all cores) ----
                ema_cs_sb = fp.tile([128, KC], F32)
                nc.sync.dma_start(ema_cs_sb[:], ema_cs_d.ap().rearrange("(t p) -> p t", p=128))
                ema_cs_scaled = fp.tile([128, KC], F32)
                nc.vector.tensor_scalar(out=ema_cs_scaled[:], in0=ema_cs_sb[:],
                                        scalar1=DECAY, scalar2=None,
                                        op0=mybir.AluOpType.mult)

                dwc = [fp.tile([128, 258], F32, name=f"dwc{t}", tag=f"dwc{t}")
                       for t in range(KC)]
                for t in range(KC):
                    nc.sync.dma_start(dwc[t][:], ar_out[t * 128:(t + 1) * 128, :])

                newcs_raw = fp.tile([128, KC], F32)
                for t in range(KC):
                    nc.vector.tensor_scalar(
                        out=newcs_raw[:, t:t + 1], in0=dwc[t][:, 256:257],
                        scalar1=1.0 - DECAY, scalar2=ema_cs_scaled[:, t:t + 1],
                        op0=mybir.AluOpType.mult, op1=mybir.AluOpType.add)
                ncs_sum = fp.tile([128, 1], F32)
                nc.vector.reduce_sum(out=ncs_sum[:], in_=newcs_raw[:], axis=mybir.AxisListType.X)
                n_tot = fp.tile([128, 1], F32)
                nc.gpsimd.partition_all_reduce(n_tot[:], ncs_sum[:],
                                               channels=128, reduce_op=bass_isa.ReduceOp.add)
                denom = fp.tile([128, 1], F32)
                nc.vector.tensor_scalar(out=denom[:], in0=n_tot[:], scalar1=float(K) * EPS,
                                        scalar2=None, op0=mybir.AluOpType.add)
                recip_d = fp.tile([128, 1], F32)
                nc.vector.reciprocal(out=recip_d[:], in_=denom[:])
                factor = fp.tile([128, 1], F32)
                nc.vector.tensor_tensor(out=factor[:], in0=n_tot[:], in1=recip_d[:],
                                        op=mybir.AluOpType.mult)
                newcs_fin = fp.tile([128, KC], F32)
                for t in range(KC):
                    nc.vector.tensor_scalar(
                        out=newcs_fin[:, t:t + 1], in0=newcs_raw[:, t:t + 1],
                        scalar1=EPS, scalar2=factor[:, 0:1],
                        op0=mybir.AluOpType.add, op1=mybir.AluOpType.mult)
                nc.sync.dma_start(new_cs_d.ap().rearrange("(t p) -> p t", p=128), newcs_fin[:])
                newcs_recip = fp.tile([128, KC], F32)
                nc.vector.reciprocal(out=newcs_recip[:], in_=newcs_fin[:])

                for t in range(KC):
                    ema_w_t = fp.tile([128, D], F32, tag="ema_w_t", bufs=2)
                    nc.sync.dma_start(ema_w_t[:], ema_w_d[t * 128:(t + 1) * 128, :])
                    a = fp.tile([128, D], F32, tag="ema_sc", bufs=2)
                    nc.vector.tensor_scalar(out=a[:], in0=ema_w_t[:], scalar1=DECAY,
                                            scalar2=None, op0=mybir.AluOpType.mult)
                    bsc = fp.tile([128, D], F32, tag="dw_sc", bufs=2)
                    nc.vector.tensor_scalar(out=bsc[:], in0=dwc[t][:, 0:256],
                                            scalar1=1.0 - DECAY, scalar2=None,
                                            op0=mybir.AluOpType.mult)
                    new_w = fp.tile([128, D], F32, tag="new_w", bufs=2)
                    nc.vector.tensor_tensor(out=new_w[:], in0=a[:], in1=bsc[:],
                                            op=mybir.AluOpType.add)
                    nc.sync.dma_start(new_ema_w_d[t * 128:(t + 1) * 128, :], new_w[:])
                    new_e = fp.tile([128, D], F32, tag="new_e", bufs=2)
                    nc.vector.tensor_scalar(out=new_e[:], in0=new_w[:],
                                            scalar1=newcs_recip[:, t:t + 1], scalar2=None,
                                            op0=mybir.AluOpType.mult)
                    nc.sync.dma_start(new_emb_d[t * 128:(t + 1) * 128, :], new_e[:])

                # loss = BETA * total / (N*D) ; N*D = 2^23, BETA=0.25 -> *2^-25
                loss_sb = fp.tile([1, 1], F32)
                nc.vector.tensor_scalar(out=loss_sb[:], in0=dwc[0][0:1, 257:258],
                                        scalar1=float(2.0 ** -25), scalar2=None,
                                        op0=mybir.AluOpType.mult)
                nc.sync.dma_start(loss_d[:], loss_sb[:])

    nc.compile()
    return nc


def _get_nc():
    global _CACHED_NC
    if _CACHED_NC is None:
        _CACHED_NC = _build_nc()
    return _CACHED_NC


def kernel(z, embedding, ema_cluster_size, ema_w):
    z = np.ascontiguousarray(np.asarray(z, dtype=np.float32))
    embedding = np.ascontiguousarray(np.asarray(embedding, dtype=np.float32))
    ema_cluster_size = np.ascontiguousarray(np.asarray(ema_cluster_size, dtype=np.float32))
    ema_w = np.ascontiguousarray(np.asarray(ema_w, dtype=np.float32))

    nc = _get_nc()
    in_maps = []
    for c in range(N_CORES):
        zs = z[c * B_LOC:(c + 1) * B_LOC].reshape(B_LOC, D, HW)
        in_maps.append({
            "z": np.ascontiguousarray(zs),
            "emb": embedding,
            "ema_cs": ema_cluster_size,
            "ema_w": ema_w,
        })
    res = run_bass_kernel_spmd(nc, in_maps, core_ids=list(range(N_CORES))).results

    zq = np.concatenate([r["zq"] for r in res], axis=0).reshape(B, D, 32, 32)
    idx = np.concatenate([r["idx"] for r in res], axis=0).astype(np.int32)
    loss = np.float32(res[0]["loss"][0, 0])
    new_emb = res[0]["new_emb"]
    new_cs = res[0]["new_cs"]
    new_ema_w = res[0]["new_ema_w"]
    return zq, idx, loss, new_emb, new_cs, new_ema_w
